# revision 7
# baseline (speedup 1.0000x reference)
"""Causal self-attention (B=2, T=2048, D=1024, H=16) on 8 trn2 NeuronCores.

Sharding: core = (batch b, head-group g) with 4 heads per group.
Each core computes its heads' full attention plus its slice of the output
projection; the host sums the 4 per-group partial outputs per batch.

Layout: scores are computed transposed ([s, t], keys on partitions) so
softmax's sum over s comes free from an extra all-ones column in the attn@v
stationary operand, and the attention output lands pre-transposed ([hd, t])
which is exactly the lhsT layout the output projection needs.

Precision: x and all weights are host-cast to bf16 (halves the input DMA
and SBUF footprint; PE rate is identical); scores/attention stay in
fp32r/fp32.

Schedule: every engine queue is in-order, so emission order IS the overlap
schedule. The kernel is ScalarE(exp)-bound during attention and PE-bound in
the projections; stage-1 of chunk j+1 and out-projection of chunk j-1 are
emitted interleaved between attention iterations of chunk j, and attn@v
trails the scores matmuls by 2 key-tiles so exp/affine latency never stalls
PE. Pools persist across reps so back-to-back kernels pipeline (xw has 2
buffers: the next rep's x/weight loads overlap this rep's compute).
"""

import numpy as np
import ml_dtypes
from contextlib import ExitStack

import concourse.bass as bass
import concourse.tile as tile
from concourse import mybir
from concourse.bass_utils import run_bass_kernel_spmd
from concourse.vector_clock import ScopedClock, VectorClock

B, T, D, H = 2, 2048, 1024, 16
HD = D // H            # 64
HG = 4                 # heads per core
GD = HG * HD           # 256, per-core projection width
NCk = D // 128         # 8 contraction chunks over D
NS = T // 128          # 16 s-tiles
TCH = 512              # t-chunk width
NJ = T // TCH          # 4 t-chunks
F32 = mybir.dt.float32
F32R = mybir.dt.float32r  # TF32-class matmul inputs: 4x PE throughput vs fp32
BF16 = mybir.dt.bfloat16
AV_LAG = 2             # attn@v trails scores by this many key-tiles

# ---------------------------------------------------------------------------
# Walrus on this image accepts only 1 sync-wait slot on regular instructions
# (2 on EventSemaphore), but Tile emits multi-wait instructions. Split excess
# waits onto EventSemaphore instructions inserted before, same engine.


def _drain_and_barrier_split(self, tick_clock, wait_clock):
    vc = tick_clock.global_clock
    n = len(vc)
    procs = [(p, vc[p]) for p in range(n) if vc[p] > 0]
    for k in range(len(procs)):
        vec = [0] * n
        p, t = procs[k]
        vec[p] = t
        d = self.nc.sync.drain()
        wait_clock.add_sem_waits(d.ins, ScopedClock({None: VectorClock(vec)}))
    self.nc.all_engine_barrier()
    assert self.sems is not None
    popped = self.nc._tile_sem_poison_stack.pop()
    assert popped is self._sem_poison
    self.nc.clear_and_free_semaphores(list(self.sems.allocated().values()))
    self.nc.all_engine_barrier()


def _split_waits(ordered):
    for bb_name, insts in ordered.items():
        out = []
        for inst in insts:
            si = inst.sync_info
            waits = list(si.on_wait) if si is not None and si.on_wait else []
            if len(waits) > 1:
                extra, keep = waits[:-1], waits[-1:]
                for k in range(0, len(extra), 2):
                    ev = mybir.InstEventSemaphore(
                        name=f"{inst.name}-sw{k}", ins=[], outs=[]
                    )
                    ev.engine = inst.engine
                    ev.debug = inst.debug
                    ev.sync_info = mybir.SyncInfo(
                        on_update=[], on_wait=extra[k : k + 2]
                    )
                    out.append(ev)
                inst.sync_info = mybir.SyncInfo(
                    on_update=list(si.on_update) if si.on_update else [],
                    on_wait=keep,
                )
            out.append(inst)
        ordered[bb_name] = out


_patched = False


def _apply_patches():
    global _patched
    if _patched:
        return
    _patched = True
    tile.TileContext._drain_and_barrier = _drain_and_barrier_split
    orig_lower = tile.TileContext._lower_ordered_insts

    def lower_with_split(self, ordered):
        _split_waits(ordered)
        return orig_lower(self, ordered)

    tile.TileContext._lower_ordered_insts = lower_with_split


# ---------------------------------------------------------------------------


def _build_nc(reps=1):
    nc = bass.Bass(trn_type="TRN2", debug=False)
    xT = nc.dram_tensor("xT", [D, T], BF16, kind="ExternalInput").ap()
    wq = nc.dram_tensor("wq", [D, GD], BF16, kind="ExternalInput").ap()
    wk = nc.dram_tensor("wk", [D, GD], BF16, kind="ExternalInput").ap()
    wv = nc.dram_tensor("wv", [D, GD], BF16, kind="ExternalInput").ap()
    wo = nc.dram_tensor("wo", [GD, D], BF16, kind="ExternalInput").ap()
    vone = nc.dram_tensor("vone", [128, HD], F32R, kind="ExternalInput").ap()
    pat2 = nc.dram_tensor("pat2", [33, 128], F32R, kind="ExternalInput").ap()
    y = nc.dram_tensor("y", [T, D], F32, kind="ExternalOutput").ap()

    xT_d = xT.rearrange("(n p) t -> n p t", p=128)     # [8, 128, 2048]
    wq_d = wq.rearrange("(n p) d -> n p d", p=128)     # [8, 128, 256]
    wk_d = wk.rearrange("(n p) d -> n p d", p=128)
    wv_d = wv.rearrange("(n p) d -> n p d", p=128)
    wo_d = wo.rearrange("(n p) d -> n p d", p=128)     # [2, 128, 1024]
    y_d = y.rearrange("(n p) d -> n p d", p=128)       # [16, 128, 1024]

    with ExitStack() as outer:
        tc = outer.enter_context(tile.TileContext(nc))
        # pools persist across reps: tag rotation double-buffers xw so the
        # next rep's loads overlap this rep's compute
        qkv = outer.enter_context(tc.tile_pool(name="qkv", bufs=1))
        xw = outer.enter_context(tc.tile_pool(name="xw", bufs=2))
        ptp = outer.enter_context(tc.tile_pool(name="ptp", bufs=2 * AV_LAG + 3))
        nrm = outer.enter_context(tc.tile_pool(name="nrm", bufs=3))
        yout = outer.enter_context(tc.tile_pool(name="yout", bufs=3))
        ps1 = outer.enter_context(tc.tile_pool(name="ps1", bufs=2, space="PSUM"))
        pso = outer.enter_context(tc.tile_pool(name="pso", bufs=3, space="PSUM"))
        psv = outer.enter_context(tc.tile_pool(name="psv", bufs=2, space="PSUM"))
        ps3 = outer.enter_context(tc.tile_pool(name="ps3", bufs=1, space="PSUM"))
        pools = (qkv, xw, ptp, nrm, yout, ps1, pso, psv, ps3)
        for _rep in range(reps):
            _one_rep(nc, pools, xT_d, wq_d, wk_d, wv_d, wo_d, y_d, vone, pat2)
    return nc


def _one_rep(nc, pools, xT_d, wq_d, wk_d, wv_d, wo_d, y_d, vone, pat2):
    qkv, xw, ptp, nrm, yout, ps1, pso, psv, ps3 = pools

    # persistent sbuf tensors (tag-stable across reps; bufs=1 -> same slot,
    # cross-rep WAR deps are range-tracked so disjoint chunks overlap)
    qT2 = [qkv.tile([128, T], F32R, tag=f"qT{m}", name=f"qT{m}") for m in range(2)]
    kT2 = [qkv.tile([128, T], F32R, tag=f"kT{m}", name=f"kT{m}") for m in range(2)]
    aoT = [qkv.tile([128, T], BF16, tag=f"aoT{m}", name=f"aoT{m}") for m in range(2)]
    vext = [
        qkv.tile([128, HG * (HD + 1)], F32R, tag=f"v{i}", name=f"v{i}")
        for i in range(NS)
    ]
    wo_sb = [
        qkv.tile([128, D], BF16, tag=f"wo{m}", name=f"wo{m}") for m in range(2)
    ]
    ones_sb = qkv.tile([128, HD], F32R, tag="ones", name="ones_sb")
    pat2_sb = qkv.tile([33, 128], F32R, tag="pat2", name="pat2_sb")

    xT_sb = [
        xw.tile([128, T], BF16, tag=f"xT{c}", name=f"xT{c}") for c in range(NCk)
    ]
    wq_sb = [
        xw.tile([128, GD], BF16, tag=f"wq{c}", name=f"wq{c}") for c in range(NCk)
    ]
    wk_sb = [
        xw.tile([128, GD], BF16, tag=f"wk{c}", name=f"wk{c}") for c in range(NCk)
    ]
    wv_sb = [
        xw.tile([128, GD], BF16, tag=f"wv{c}", name=f"wv{c}") for c in range(NCk)
    ]
    # x and the q-weights gate the first matmul group: load them first
    for c in range(NCk):
        nc.sync.dma_start(out=xT_sb[c][:], in_=xT_d[c])
    for c in range(NCk):
        nc.sync.dma_start(out=wq_sb[c][:], in_=wq_d[c])
    for c in range(NCk):
        nc.sync.dma_start(out=wk_sb[c][:], in_=wk_d[c])
    for c in range(NCk):
        nc.sync.dma_start(out=wv_sb[c][:], in_=wv_d[c])
    nc.sync.dma_start(out=ones_sb[:], in_=vone)
    nc.sync.dma_start(out=pat2_sb[:], in_=pat2)
    for m in range(2):
        nc.sync.dma_start(out=wo_sb[m][:], in_=wo_d[m])

    # ---- work-item generators --------------------------------------------
    def stage1_items(j):
        """8 items: 4 q/k accumulation groups + 4 v s-tiles."""
        items = []
        for dst, w_sb in ((qT2, wq_sb), (kT2, wk_sb)):
            for m in range(2):
                def qk_group(dst=dst, w_sb=w_sb, m=m):
                    acc = ps1.tile([128, TCH], F32, tag="ps1", name="acc")
                    for c in range(NCk):
                        nc.tensor.matmul(
                            acc[:],
                            w_sb[c][:, m * 128 : (m + 1) * 128],
                            xT_sb[c][:, j * TCH : (j + 1) * TCH],
                            start=(c == 0),
                            stop=(c == NCk - 1),
                        )
                    nc.vector.tensor_copy(
                        dst[m][:, j * TCH : (j + 1) * TCH], acc[:]
                    )
                items.append(qk_group)
        for i in range(4 * j, 4 * j + 4):
            def v_tile(i=i):
                acc = ps1.tile([128, GD], F32, tag="ps1", name="accv")
                for c in range(NCk):
                    nc.tensor.matmul(
                        acc[:],
                        xT_sb[c][:, i * 128 : (i + 1) * 128],
                        wv_sb[c][:],
                        start=(c == 0),
                        stop=(c == NCk - 1),
                    )
                v_view = vext[i].rearrange("p (h e) -> p h e", e=HD + 1)
                nc.vector.tensor_copy(
                    v_view[:, :, 0:HD], acc.rearrange("p (h e) -> p h e", e=HD)
                )
                nc.vector.tensor_copy(
                    v_view[:, :, HD : HD + 1],
                    ones_sb[:, 0:HG].rearrange("p (h o) -> p h o", o=1),
                )
            items.append(v_tile)
        return items

    def outproj_items(j):
        """4 items: one per 128-row t-tile of chunk j."""
        items = []
        for tt in range(4 * j, 4 * j + 4):
            def oproj(tt=tt):
                y_sb = yout.tile([128, D], F32, tag="ysb", name="y_sb")
                for e in range(2):
                    acc = ps3.tile([128, TCH], F32, tag="ps3", name="acc3")
                    for m in range(2):
                        nc.tensor.matmul(
                            acc[:],
                            aoT[m][:, tt * 128 : (tt + 1) * 128],
                            wo_sb[m][:, e * TCH : (e + 1) * TCH],
                            start=(m == 0),
                            stop=(m == 1),
                        )
                    nc.vector.tensor_copy(
                        y_sb[:, e * TCH : (e + 1) * TCH], acc[:]
                    )
                nc.gpsimd.dma_start(out=y_d[tt][:], in_=y_sb[:])
            items.append(oproj)
        return items

    def attn_items(j):
        """Attention for chunk j, software-pipelined: iteration i emits the
        two scores matmuls for key-tile i (feeding ScalarE exp + the Pool
        affine_select on diagonal tiles), then the attn@v matmuls for
        key-tile i-AV_LAG, so exp/affine latency hides behind PE work.
        The 1/d normalization is split in two items (ScalarE chain, then
        PE broadcast + DVE applies) so a filler can sit between them."""
        n_i = 4 * j + 4
        per_p = []
        for p in range(2):
            outp_box = {}
            pt_box = {}
            items = []

            def sc_pair(p, i, outp_box, pt_box):
                c0 = max(0, 128 * i - TCH * j)
                c0n = min(c0, TCH - 256)
                if i == 0:
                    for hp in range(2):
                        outp_box[hp] = psv.tile(
                            [HD + 1, TCH], F32, tag="outp", name=f"outp{hp}"
                        )
                for hp in range(2):
                    sc = pso.tile([128, TCH], F32, tag="sc", name="sc")
                    pt = ptp.tile([128, TCH], F32R, tag="pt", name="pt")
                    pt_box[(i, hp)] = pt
                    nc.tensor.matmul(
                        sc[:, c0n:TCH],
                        kT2[p][hp * 64 : hp * 64 + 64, i * 128 : (i + 1) * 128],
                        qT2[p][hp * 64 : hp * 64 + 64, j * TCH + c0n : (j + 1) * TCH],
                        start=True,
                        stop=True,
                    )
                    # exp only over the causally-valid range; the stale
                    # [c0n:c0) region is zeroed by the affine_select before
                    # attn@v reads it
                    nc.scalar.activation(
                        pt[:, c0:TCH],
                        sc[:, c0:TCH],
                        mybir.ActivationFunctionType.Exp,
                        scale=1.0 / np.sqrt(HD),
                    )
                    if i // 4 == j:
                        me = min(c0 + 128, TCH)
                        nc.gpsimd.affine_select(
                            out=pt[:, c0n:me],
                            in_=pt[:, c0n:me],
                            compare_op=mybir.AluOpType.is_ge,
                            fill=0.0,
                            base=j * TCH + c0n - i * 128,
                            pattern=[[1, me - c0n]],
                            channel_multiplier=-1,
                        )

            def av_pair(p, i, outp_box, pt_box):
                c0 = max(0, 128 * i - TCH * j)
                c0n = min(c0, TCH - 256)
                for hp in range(2):
                    hl = 2 * p + hp
                    nc.tensor.matmul(
                        outp_box[hp][:, c0n:TCH],
                        vext[i][:, hl * (HD + 1) : (hl + 1) * (HD + 1)],
                        pt_box.pop((i, hp))[:, c0n:TCH],
                        start=(i == 0),
                        stop=(i == n_i - 1),
                    )

            for i in range(n_i):
                def iter_item(p=p, i=i, outp_box=outp_box, pt_box=pt_box):
                    sc_pair(p, i, outp_box, pt_box)
                    if i >= AV_LAG:
                        av_pair(p, i - AV_LAG, outp_box, pt_box)
                items.append(iter_item)

            for i in range(max(0, n_i - AV_LAG), n_i):
                def tail_av(p=p, i=i, outp_box=outp_box, pt_box=pt_box):
                    av_pair(p, i, outp_box, pt_box)
                items.append(tail_av)

            def norm_a(p=p, outp_box=outp_box):
                # batched 1/d: both heads' ones-row denominators gathered to
                # partitions 0 and 32 (engine APs need 32-aligned bases),
                # then one Ln + one Exp covers both. The memset keeps the
                # untouched rows finite so the pattern matmul's 0-rows don't
                # multiply NaNs.
                den = nrm.tile([33, TCH], F32, tag="den", name="den")
                nc.vector.memset(den[:], 1.0)
                nc.vector.tensor_copy(den[0:1, :], outp_box[0][HD : HD + 1, :])
                nc.vector.tensor_copy(den[32:33, :], outp_box[1][HD : HD + 1, :])
                lnd = nrm.tile([33, TCH], F32, tag="lnd", name="lnd")
                nc.scalar.activation(
                    lnd[:], den[:], mybir.ActivationFunctionType.Ln
                )
                rec2 = nrm.tile([33, TCH], F32R, tag="rec2", name="rec2")
                nc.scalar.activation(
                    rec2[:], lnd[:], mybir.ActivationFunctionType.Exp, scale=-1.0
                )
                outp_box["rec2"] = rec2

            def norm_b(p=p, outp_box=outp_box, j=j):
                # PE broadcast via the block-ones pattern, then DVE applies
                # 1/d and writes the bf16 aoT slices
                bc = ps1.tile([128, TCH], F32, tag="ps1", name="bc")
                nc.tensor.matmul(
                    bc[:], pat2_sb[:], outp_box["rec2"][:], start=True, stop=True
                )
                bc_sb = nrm.tile([128, TCH], F32, tag="bcsb", name="bc_sb")
                nc.vector.tensor_copy(bc_sb[:], bc[:])
                for hp in range(2):
                    nc.vector.tensor_mul(
                        aoT[p][hp * 64 : hp * 64 + 64, j * TCH : (j + 1) * TCH],
                        outp_box[hp][0:HD, :],
                        bc_sb[hp * 64 : hp * 64 + 64, :],
                    )
            per_p.append((items, [norm_a, norm_b]))
        # p0's norm items slot between p1's first iterations so the ScalarE
        # 1/d chain and its PE broadcast never stall the in-order PE queue;
        # p1's norms are returned as carry for the next phase's stream
        p0_items, p0_norms = per_p[0]
        p1_items, p1_norms = per_p[1]
        items = (
            p0_items
            + [p1_items[0], p0_norms[0], p1_items[1], p0_norms[1]]
            + p1_items[2:]
        )
        return items, p1_norms

    # ---- emission schedule ------------------------------------------------
    def interleave(main, fillers):
        """Emit main items with fillers spread evenly between them."""
        if not fillers:
            for it in main:
                it()
            return
        n, f = len(main), len(fillers)
        fi = 0
        for k, it in enumerate(main):
            it()
            want = (k + 1) * f // n
            while fi < want:
                fillers[fi]()
                fi += 1
        while fi < f:
            fillers[fi]()
            fi += 1

    for it in stage1_items(0):
        it()
    pending_op = []
    carry = []
    for j in range(NJ):
        fillers = []
        if j + 1 < NJ:
            fillers.extend(stage1_items(j + 1))
        if j > 0:
            pending_op.extend(outproj_items(j - 1))
        # hold a couple of out-proj items back for the filler-poor last chunk
        take = len(pending_op) if j == NJ - 1 else max(0, len(pending_op) - 2)
        fillers.extend(pending_op[:take])
        pending_op = pending_op[take:]
        items, new_carry = attn_items(j)
        if carry:
            items = [items[0], carry[0], items[1], carry[1]] + items[2:]
        interleave(items, fillers)
        carry = new_carry
    for it in carry:
        it()
    for it in pending_op:
        it()
    for it in outproj_items(NJ - 1):
        it()


_nc_cache = None


def _get_nc():
    global _nc_cache
    if _nc_cache is None:
        _apply_patches()
        _nc_cache = _build_nc()
    return _nc_cache


def _pat2_np():
    pat = np.zeros((33, 128), np.float32)
    pat[0, 0:64] = 1.0
    pat[32, 64:128] = 1.0
    return pat


def _bf16(a):
    return np.ascontiguousarray(a).astype(ml_dtypes.bfloat16)


def kernel(x, Wq, Wk, Wv, Wo, mask, _want_results=False, _trace=False):
    x = np.asarray(x, dtype=np.float32)
    Wq = np.asarray(Wq, dtype=np.float32)
    Wk = np.asarray(Wk, dtype=np.float32)
    Wv = np.asarray(Wv, dtype=np.float32)
    Wo = np.asarray(Wo, dtype=np.float32)

    nc = _get_nc()
    in_maps = []
    for core in range(8):
        b, g = divmod(core, HG)
        sl = slice(g * GD, (g + 1) * GD)
        in_maps.append(
            {
                "xT": _bf16(x[b].T),
                "wq": _bf16(Wq[sl, :].T),
                "wk": _bf16(Wk[sl, :].T),
                "wv": _bf16(Wv[sl, :].T),
                "wo": _bf16(Wo[:, sl].T),
                "vone": np.ones((128, HD), np.float32),
                "pat2": _pat2_np(),
            }
        )
    res = run_bass_kernel_spmd(
        nc, in_maps, core_ids=list(range(8)), trace=_trace
    )
    y = np.zeros((B, T, D), dtype=np.float32)
    for core in range(8):
        b = core // HG
        y[b] += res.results[core]["y"]
    if _want_results:
        return y, res
    return y


# revision 12
# speedup vs baseline: 1.0898x; 1.0898x over previous
"""Causal self-attention (B=2, T=2048, D=1024, H=16) on 8 trn2 NeuronCores.

Sharding: core = (batch b, head-group g) with 4 heads per group.
Each core computes its heads' full attention plus its slice of the output
projection; the host sums the 4 per-group partial outputs per batch.

Layout: scores are computed transposed ([s, t], keys on partitions) so
softmax's sum over s comes free from an extra all-ones column in the attn@v
stationary operand, and the attention output lands pre-transposed ([hd, t])
which is exactly the lhsT layout the output projection needs.

Precision: x and all weights are host-cast to bf16 (halves the input DMA
and SBUF footprint; PE rate is identical); scores/attention stay in
fp32r/fp32.

Schedule: every engine queue is in-order, so emission order IS the overlap
schedule. The kernel is ScalarE(exp)-bound during attention and PE-bound in
the projections; stage-1 of chunk j+1 and out-projection of chunk j-1 are
emitted interleaved between attention iterations of chunk j, and attn@v
trails the scores matmuls by 2 key-tiles so exp/affine latency never stalls
PE. Pools persist across reps so back-to-back kernels pipeline (xw has 2
buffers: the next rep's x/weight loads overlap this rep's compute).
"""

import numpy as np
import ml_dtypes
from contextlib import ExitStack

import concourse.bass as bass
import concourse.tile as tile
from concourse import mybir
from concourse.bass_utils import run_bass_kernel_spmd
from concourse.vector_clock import ScopedClock, VectorClock

B, T, D, H = 2, 2048, 1024, 16
HD = D // H            # 64
HG = 4                 # heads per core
GD = HG * HD           # 256, per-core projection width
NCk = D // 128         # 8 contraction chunks over D
NS = T // 128          # 16 s-tiles
TCH = 512              # t-chunk width
NJ = T // TCH          # 4 t-chunks
F32 = mybir.dt.float32
F32R = mybir.dt.float32r  # TF32-class matmul inputs: 4x PE throughput vs fp32
BF16 = mybir.dt.bfloat16
AV_LAG = 3             # attn@v trails scores by this many key-tiles
NA_POS = 1             # items into the next stream before norm_a is emitted
NB_POS = 4             # items into the next stream before norm_b is emitted

# ---------------------------------------------------------------------------
# Walrus on this image accepts only 1 sync-wait slot on regular instructions
# (2 on EventSemaphore), but Tile emits multi-wait instructions. Split excess
# waits onto EventSemaphore instructions inserted before, same engine.


def _drain_and_barrier_split(self, tick_clock, wait_clock):
    vc = tick_clock.global_clock
    n = len(vc)
    procs = [(p, vc[p]) for p in range(n) if vc[p] > 0]
    for k in range(len(procs)):
        vec = [0] * n
        p, t = procs[k]
        vec[p] = t
        d = self.nc.sync.drain()
        wait_clock.add_sem_waits(d.ins, ScopedClock({None: VectorClock(vec)}))
    self.nc.all_engine_barrier()
    assert self.sems is not None
    popped = self.nc._tile_sem_poison_stack.pop()
    assert popped is self._sem_poison
    self.nc.clear_and_free_semaphores(list(self.sems.allocated().values()))
    self.nc.all_engine_barrier()


def _split_waits(ordered):
    for bb_name, insts in ordered.items():
        out = []
        for inst in insts:
            si = inst.sync_info
            waits = list(si.on_wait) if si is not None and si.on_wait else []
            if len(waits) > 1:
                extra, keep = waits[:-1], waits[-1:]
                for k in range(0, len(extra), 2):
                    ev = mybir.InstEventSemaphore(
                        name=f"{inst.name}-sw{k}", ins=[], outs=[]
                    )
                    ev.engine = inst.engine
                    ev.debug = inst.debug
                    ev.sync_info = mybir.SyncInfo(
                        on_update=[], on_wait=extra[k : k + 2]
                    )
                    out.append(ev)
                inst.sync_info = mybir.SyncInfo(
                    on_update=list(si.on_update) if si.on_update else [],
                    on_wait=keep,
                )
            out.append(inst)
        ordered[bb_name] = out


_patched = False


def _apply_patches():
    global _patched
    if _patched:
        return
    _patched = True
    tile.TileContext._drain_and_barrier = _drain_and_barrier_split
    orig_lower = tile.TileContext._lower_ordered_insts

    def lower_with_split(self, ordered):
        _split_waits(ordered)
        return orig_lower(self, ordered)

    tile.TileContext._lower_ordered_insts = lower_with_split


# ---------------------------------------------------------------------------


def _build_nc(reps=1):
    nc = bass.Bass(trn_type="TRN2", debug=False)
    xT = nc.dram_tensor("xT", [D, T], BF16, kind="ExternalInput").ap()
    wq = nc.dram_tensor("wq", [D, GD], BF16, kind="ExternalInput").ap()
    wk = nc.dram_tensor("wk", [D, GD], BF16, kind="ExternalInput").ap()
    wv = nc.dram_tensor("wv", [D, GD], BF16, kind="ExternalInput").ap()
    wo = nc.dram_tensor("wo", [GD, D], BF16, kind="ExternalInput").ap()
    vone = nc.dram_tensor("vone", [128, HD], F32R, kind="ExternalInput").ap()
    pat2 = nc.dram_tensor("pat2", [33, 128], F32R, kind="ExternalInput").ap()
    y = nc.dram_tensor("y", [T, D], F32, kind="ExternalOutput").ap()

    xT_d = xT.rearrange("(n p) t -> n p t", p=128)     # [8, 128, 2048]
    wq_d = wq.rearrange("(n p) d -> n p d", p=128)     # [8, 128, 256]
    wk_d = wk.rearrange("(n p) d -> n p d", p=128)
    wv_d = wv.rearrange("(n p) d -> n p d", p=128)
    wo_d = wo.rearrange("(n p) d -> n p d", p=128)     # [2, 128, 1024]
    y_d = y.rearrange("(n p) d -> n p d", p=128)       # [16, 128, 1024]

    with ExitStack() as outer:
        tc = outer.enter_context(tile.TileContext(nc))
        # pools persist across reps: tag rotation double-buffers xw so the
        # next rep's loads overlap this rep's compute
        qkv = outer.enter_context(tc.tile_pool(name="qkv", bufs=1))
        xw = outer.enter_context(tc.tile_pool(name="xw", bufs=2))
        ptp = outer.enter_context(tc.tile_pool(name="ptp", bufs=2 * AV_LAG + 3))
        nrm = outer.enter_context(tc.tile_pool(name="nrm", bufs=3))
        yout = outer.enter_context(tc.tile_pool(name="yout", bufs=3))
        ps1 = outer.enter_context(tc.tile_pool(name="ps1", bufs=2, space="PSUM"))
        pso = outer.enter_context(tc.tile_pool(name="pso", bufs=3, space="PSUM"))
        psv = outer.enter_context(tc.tile_pool(name="psv", bufs=2, space="PSUM"))
        ps3 = outer.enter_context(tc.tile_pool(name="ps3", bufs=1, space="PSUM"))
        pools = (qkv, xw, ptp, nrm, yout, ps1, pso, psv, ps3)
        for _rep in range(reps):
            _one_rep(nc, pools, xT_d, wq_d, wk_d, wv_d, wo_d, y_d, vone, pat2)
    return nc


def _one_rep(nc, pools, xT_d, wq_d, wk_d, wv_d, wo_d, y_d, vone, pat2):
    qkv, xw, ptp, nrm, yout, ps1, pso, psv, ps3 = pools

    # persistent sbuf tensors (tag-stable across reps; bufs=1 -> same slot,
    # cross-rep WAR deps are range-tracked so disjoint chunks overlap)
    qT2 = [qkv.tile([128, T], F32R, tag=f"qT{m}", name=f"qT{m}") for m in range(2)]
    kT2 = [qkv.tile([128, T], F32R, tag=f"kT{m}", name=f"kT{m}") for m in range(2)]
    aoT = [qkv.tile([128, T], BF16, tag=f"aoT{m}", name=f"aoT{m}") for m in range(2)]
    vext = [
        qkv.tile([128, HG * (HD + 1)], F32R, tag=f"v{i}", name=f"v{i}")
        for i in range(NS)
    ]
    wo_sb = [
        qkv.tile([128, D], BF16, tag=f"wo{m}", name=f"wo{m}") for m in range(2)
    ]
    ones_sb = qkv.tile([128, HD], F32R, tag="ones", name="ones_sb")
    pat2_sb = qkv.tile([33, 128], F32R, tag="pat2", name="pat2_sb")

    xT_sb = [
        xw.tile([128, T], BF16, tag=f"xT{c}", name=f"xT{c}") for c in range(NCk)
    ]
    wq_sb = [
        xw.tile([128, GD], BF16, tag=f"wq{c}", name=f"wq{c}") for c in range(NCk)
    ]
    wk_sb = [
        xw.tile([128, GD], BF16, tag=f"wk{c}", name=f"wk{c}") for c in range(NCk)
    ]
    wv_sb = [
        xw.tile([128, GD], BF16, tag=f"wv{c}", name=f"wv{c}") for c in range(NCk)
    ]
    # x and the q-weights gate the first matmul group: load them first
    for c in range(NCk):
        nc.sync.dma_start(out=xT_sb[c][:], in_=xT_d[c])
    for c in range(NCk):
        nc.sync.dma_start(out=wq_sb[c][:], in_=wq_d[c])
    for c in range(NCk):
        nc.sync.dma_start(out=wk_sb[c][:], in_=wk_d[c])
    for c in range(NCk):
        nc.sync.dma_start(out=wv_sb[c][:], in_=wv_d[c])
    nc.sync.dma_start(out=ones_sb[:], in_=vone)
    nc.sync.dma_start(out=pat2_sb[:], in_=pat2)
    for m in range(2):
        nc.sync.dma_start(out=wo_sb[m][:], in_=wo_d[m])

    # ---- work-item generators --------------------------------------------
    def stage1_items(j):
        """8 items: 4 q/k accumulation groups + 4 v s-tiles."""
        items = []
        for dst, w_sb in ((qT2, wq_sb), (kT2, wk_sb)):
            for m in range(2):
                box = {}

                def qk_half_a(w_sb=w_sb, m=m, box=box):
                    box["acc"] = ps1.tile([128, TCH], F32, tag="ps1", name="acc")
                    for c in range(NCk // 2):
                        nc.tensor.matmul(
                            box["acc"][:],
                            w_sb[c][:, m * 128 : (m + 1) * 128],
                            xT_sb[c][:, j * TCH : (j + 1) * TCH],
                            start=(c == 0),
                            stop=False,
                        )

                def qk_half_b(dst=dst, w_sb=w_sb, m=m, box=box):
                    for c in range(NCk // 2, NCk):
                        nc.tensor.matmul(
                            box["acc"][:],
                            w_sb[c][:, m * 128 : (m + 1) * 128],
                            xT_sb[c][:, j * TCH : (j + 1) * TCH],
                            start=False,
                            stop=(c == NCk - 1),
                        )
                    nc.vector.tensor_copy(
                        dst[m][:, j * TCH : (j + 1) * TCH], box["acc"][:]
                    )
                items.append(qk_half_a)
                items.append(qk_half_b)
        for i in range(4 * j, 4 * j + 4):
            vbox = {}

            def v_half_a(i=i, vbox=vbox):
                vbox["acc"] = ps1.tile([128, GD], F32, tag="ps1", name="accv")
                for c in range(NCk // 2):
                    nc.tensor.matmul(
                        vbox["acc"][:],
                        xT_sb[c][:, i * 128 : (i + 1) * 128],
                        wv_sb[c][:],
                        start=(c == 0),
                        stop=False,
                    )

            def v_half_b(i=i, vbox=vbox):
                acc = vbox["acc"]
                for c in range(NCk // 2, NCk):
                    nc.tensor.matmul(
                        acc[:],
                        xT_sb[c][:, i * 128 : (i + 1) * 128],
                        wv_sb[c][:],
                        start=False,
                        stop=(c == NCk - 1),
                    )
                v_view = vext[i].rearrange("p (h e) -> p h e", e=HD + 1)
                nc.vector.tensor_copy(
                    v_view[:, :, 0:HD], acc.rearrange("p (h e) -> p h e", e=HD)
                )
                nc.vector.tensor_copy(
                    v_view[:, :, HD : HD + 1],
                    ones_sb[:, 0:HG].rearrange("p (h o) -> p h o", o=1),
                )
            items.append(v_half_a)
            items.append(v_half_b)
        return items

    def outproj_items(j):
        """4 items: one per 128-row t-tile of chunk j."""
        items = []
        for tt in range(4 * j, 4 * j + 4):
            def oproj(tt=tt):
                y_sb = yout.tile([128, D], F32, tag="ysb", name="y_sb")
                for e in range(2):
                    acc = ps3.tile([128, TCH], F32, tag="ps3", name="acc3")
                    for m in range(2):
                        nc.tensor.matmul(
                            acc[:],
                            aoT[m][:, tt * 128 : (tt + 1) * 128],
                            wo_sb[m][:, e * TCH : (e + 1) * TCH],
                            start=(m == 0),
                            stop=(m == 1),
                        )
                    nc.vector.tensor_copy(
                        y_sb[:, e * TCH : (e + 1) * TCH], acc[:]
                    )
                nc.gpsimd.dma_start(out=y_d[tt][:], in_=y_sb[:])
            items.append(oproj)
        return items

    def attn_items(j):
        """Attention for chunk j, software-pipelined: iteration i emits the
        two scores matmuls for key-tile i (feeding ScalarE exp + the Pool
        affine_select on diagonal tiles), then the attn@v matmuls for
        key-tile i-AV_LAG, so exp/affine latency hides behind PE work.
        The 1/d normalization is split in two items (ScalarE chain, then
        PE broadcast + DVE applies) so a filler can sit between them."""
        n_i = 4 * j + 4
        per_p = []
        for p in range(2):
            outp_box = {}
            pt_box = {}
            items = []

            def sc_pair(p, i, outp_box, pt_box):
                c0 = max(0, 128 * i - TCH * j)
                c0n = min(c0, TCH - 256)
                if i == 0:
                    for hp in range(2):
                        outp_box[hp] = psv.tile(
                            [HD + 1, TCH], F32, tag="outp", name=f"outp{hp}"
                        )
                for hp in range(2):
                    sc = pso.tile([128, TCH], F32, tag="sc", name="sc")
                    pt = ptp.tile([128, TCH], F32R, tag="pt", name="pt")
                    pt_box[(i, hp)] = pt
                    nc.tensor.matmul(
                        sc[:, c0n:TCH],
                        kT2[p][hp * 64 : hp * 64 + 64, i * 128 : (i + 1) * 128],
                        qT2[p][hp * 64 : hp * 64 + 64, j * TCH + c0n : (j + 1) * TCH],
                        start=True,
                        stop=True,
                    )
                    # exp only over the causally-valid range; the stale
                    # [c0n:c0) region is zeroed by the affine_select before
                    # attn@v reads it
                    nc.scalar.activation(
                        pt[:, c0:TCH],
                        sc[:, c0:TCH],
                        mybir.ActivationFunctionType.Exp,
                        scale=1.0 / np.sqrt(HD),
                    )
                    if i // 4 == j:
                        me = min(c0 + 128, TCH)
                        nc.gpsimd.affine_select(
                            out=pt[:, c0n:me],
                            in_=pt[:, c0n:me],
                            compare_op=mybir.AluOpType.is_ge,
                            fill=0.0,
                            base=j * TCH + c0n - i * 128,
                            pattern=[[1, me - c0n]],
                            channel_multiplier=-1,
                        )

            def av_pair(p, i, outp_box, pt_box):
                c0 = max(0, 128 * i - TCH * j)
                c0n = min(c0, TCH - 256)
                for hp in range(2):
                    hl = 2 * p + hp
                    nc.tensor.matmul(
                        outp_box[hp][:, c0n:TCH],
                        vext[i][:, hl * (HD + 1) : (hl + 1) * (HD + 1)],
                        pt_box.pop((i, hp))[:, c0n:TCH],
                        start=(i == 0),
                        stop=(i == n_i - 1),
                    )

            for i in range(n_i):
                def iter_item(p=p, i=i, outp_box=outp_box, pt_box=pt_box):
                    sc_pair(p, i, outp_box, pt_box)
                    if i >= AV_LAG:
                        av_pair(p, i - AV_LAG, outp_box, pt_box)
                items.append(iter_item)

            for i in range(max(0, n_i - AV_LAG), n_i):
                def tail_av(p=p, i=i, outp_box=outp_box, pt_box=pt_box):
                    av_pair(p, i, outp_box, pt_box)
                items.append(tail_av)

            def norm_a(p=p, outp_box=outp_box):
                # batched 1/d: both heads' ones-row denominators gathered to
                # partitions 0 and 32 (engine APs need 32-aligned bases),
                # then one Ln + one Exp covers both. The memset keeps the
                # untouched rows finite so the pattern matmul's 0-rows don't
                # multiply NaNs.
                den = nrm.tile([33, TCH], F32, tag="den", name="den")
                nc.vector.memset(den[:], 1.0)
                nc.vector.tensor_copy(den[0:1, :], outp_box[0][HD : HD + 1, :])
                nc.vector.tensor_copy(den[32:33, :], outp_box[1][HD : HD + 1, :])
                lnd = nrm.tile([33, TCH], F32, tag="lnd", name="lnd")
                nc.scalar.activation(
                    lnd[:], den[:], mybir.ActivationFunctionType.Ln
                )
                rec2 = nrm.tile([33, TCH], F32R, tag="rec2", name="rec2")
                nc.scalar.activation(
                    rec2[:], lnd[:], mybir.ActivationFunctionType.Exp, scale=-1.0
                )
                outp_box["rec2"] = rec2

            def norm_b(p=p, outp_box=outp_box, j=j):
                # PE broadcast via the block-ones pattern, then DVE applies
                # 1/d and writes the bf16 aoT slices
                bc = pso.tile([128, TCH], F32, tag="sc", name="bc")
                nc.tensor.matmul(
                    bc[:], pat2_sb[:], outp_box["rec2"][:], start=True, stop=True
                )
                bc_sb = nrm.tile([128, TCH], F32, tag="bcsb", name="bc_sb")
                nc.vector.tensor_copy(bc_sb[:], bc[:])
                for hp in range(2):
                    nc.vector.tensor_mul(
                        aoT[p][hp * 64 : hp * 64 + 64, j * TCH : (j + 1) * TCH],
                        outp_box[hp][0:HD, :],
                        bc_sb[hp * 64 : hp * 64 + 64, :],
                    )
            per_p.append((items, [norm_a, norm_b]))
        # norm_a goes right after p0's tail (its Ln enters the Act queue
        # before p1's exps flood it); norm_b three items later so the PE
        # broadcast never waits on the ScalarE chain. p1's norms are carried
        # into the next phase's stream the same way.
        p0_items, p0_norms = per_p[0]
        p1_items, p1_norms = per_p[1]
        items = list(p0_items)
        rest = list(p1_items)
        items += rest[:NA_POS] + [p0_norms[0]]
        rest = rest[NA_POS:]
        items += rest[: NB_POS - NA_POS] + [p0_norms[1]]
        items += rest[NB_POS - NA_POS :]
        return items, p1_norms

    # ---- emission schedule ------------------------------------------------
    def interleave(main, fillers):
        """Emit main items with fillers spread evenly between them."""
        if not fillers:
            for it in main:
                it()
            return
        n, f = len(main), len(fillers)
        fi = 0
        for k, it in enumerate(main):
            it()
            want = (k + 1) * f // n
            while fi < want:
                fillers[fi]()
                fi += 1
        while fi < f:
            fillers[fi]()
            fi += 1

    for it in stage1_items(0):
        it()
    pending_op = []
    carry = []
    for j in range(NJ):
        fillers = []
        if j + 1 < NJ:
            fillers.extend(stage1_items(j + 1))
        if j > 0:
            pending_op.extend(outproj_items(j - 1))
        # hold a couple of out-proj items back for the filler-poor last chunk
        take = len(pending_op) if j == NJ - 1 else max(0, len(pending_op) - 2)
        fillers.extend(pending_op[:take])
        pending_op = pending_op[take:]
        items, new_carry = attn_items(j)
        if carry:
            items = (
                items[:NA_POS] + [carry[0]]
                + items[NA_POS:NB_POS] + [carry[1]]
                + items[NB_POS:]
            )
        interleave(items, fillers)
        carry = new_carry
    for it in carry:
        it()
    for it in pending_op:
        it()
    for it in outproj_items(NJ - 1):
        it()


_nc_cache = None


def _get_nc():
    global _nc_cache
    if _nc_cache is None:
        _apply_patches()
        _nc_cache = _build_nc()
    return _nc_cache


def _pat2_np():
    pat = np.zeros((33, 128), np.float32)
    pat[0, 0:64] = 1.0
    pat[32, 64:128] = 1.0
    return pat


def _bf16(a):
    return np.ascontiguousarray(a).astype(ml_dtypes.bfloat16)


def kernel(x, Wq, Wk, Wv, Wo, mask, _want_results=False, _trace=False):
    x = np.asarray(x, dtype=np.float32)
    Wq = np.asarray(Wq, dtype=np.float32)
    Wk = np.asarray(Wk, dtype=np.float32)
    Wv = np.asarray(Wv, dtype=np.float32)
    Wo = np.asarray(Wo, dtype=np.float32)

    nc = _get_nc()
    in_maps = []
    for core in range(8):
        b, g = divmod(core, HG)
        sl = slice(g * GD, (g + 1) * GD)
        in_maps.append(
            {
                "xT": _bf16(x[b].T),
                "wq": _bf16(Wq[sl, :].T),
                "wk": _bf16(Wk[sl, :].T),
                "wv": _bf16(Wv[sl, :].T),
                "wo": _bf16(Wo[:, sl].T),
                "vone": np.ones((128, HD), np.float32),
                "pat2": _pat2_np(),
            }
        )
    res = run_bass_kernel_spmd(
        nc, in_maps, core_ids=list(range(8)), trace=_trace
    )
    y = np.zeros((B, T, D), dtype=np.float32)
    for core in range(8):
        b = core // HG
        y[b] += res.results[core]["y"]
    if _want_results:
        return y, res
    return y
